# revision 28
# baseline (speedup 1.0000x reference)
"""Trainium2 Bass kernel for InfLoRA attention (self-contained).

Data-parallel over batch across 8 NeuronCores (2 batch elements per core).
Per-core program (all FLOPs on device):
  1. Merge LoRA weights into k/v projection weights:
     w_k_eff.T = W_qkv_k.T + Acat_k.T-contracted product (single matmul chain),
     k result goes to DRAM scratch, v result stays resident in SBUF.
  2. qkv projection in feature-major layout (weights stationary for q/k ->
     feature-major q/k; x stationary for v -> token-major v).
  3. Attention per head pair: scores^T = k^T . q (K=64) with the two heads
     packed into different PE row groups (base partitions 0/64 -> concurrent
     matmuls) sharing one 2-bank PSUM tile; exp on ACT with the 1/sqrt(hd)
     scale folded into the activation's free affine (no max-subtraction:
     |logits| < 4.2); AV matmul in bf16 with a ones-column on v giving the
     softmax denominators for free; normalization via reciprocal + DRAM-bounce
     partition broadcast. Head 0's AV accumulation is interleaved with the
     scores/exp stream so the PE has ready work while ACT is the rate limiter.
  4. Output projection (bf16) + bias, written feature-major and re-transposed
     on the host. Batches are software-pipelined: proj(b-1) is emitted after
     v(b) so batch-b PE work fills batch-(b-1)'s softmax-normalize tail.
"""

import numpy as np

B, N, C = 16, 1024, 1024
H = 16
HD = C // H  # 64
NCORES = 8
BL = B // NCORES  # 2 batch elements per core
NL = BL * N  # 2048 tokens per core
NT = N // 512  # n-tiles per batch

_BUILT = {}


def _build(rt):
    """Build + finalize the per-core Bacc program for RT = 64*(task+1)."""
    from contextlib import ExitStack

    import concourse.tile as tile
    from concourse import bacc, mybir

    f32 = mybir.dt.float32
    bf16 = mybir.dt.bfloat16
    f32r = mybir.dt.float32r
    Exp = mybir.ActivationFunctionType.Exp

    r_tiles = []
    r0 = 0
    while r0 < rt:
        r_tiles.append((r0, min(128, rt - r0)))
        r0 += 128

    nc = bacc.Bacc()

    xT = nc.declare_dram_parameter("xT", [C, NL], f32r, isOutput=False)
    wqT = nc.declare_dram_parameter("wqT", [C, C], f32r, isOutput=False)
    wkT = nc.declare_dram_parameter("wkT", [C, C], f32, isOutput=False)
    wvT = nc.declare_dram_parameter("wvT", [C, C], f32, isOutput=False)
    a_k = nc.declare_dram_parameter("a_k", [rt, C], f32r, isOutput=False)
    bkT = nc.declare_dram_parameter("bkT", [rt, C], f32r, isOutput=False)
    a_v = nc.declare_dram_parameter("a_v", [rt, C], f32r, isOutput=False)
    bvT = nc.declare_dram_parameter("bvT", [rt, C], f32r, isOutput=False)
    wpT = nc.declare_dram_parameter("wpT", [C, C], bf16, isOutput=False)
    bpr = nc.declare_dram_parameter("bpr", [C], f32, isOutput=False)
    yT = nc.declare_dram_parameter("yT", [C, NL], f32, isOutput=True)

    with tile.TileContext(nc) as tc, ExitStack() as ctx:
        ep = ctx.enter_context

        # ---- long-lived pools (created first: low addresses) ----
        const = ep(tc.tile_pool(name="const", bufs=1))
        wcvp = ep(tc.tile_pool(name="wcvp", bufs=1))
        wpp = ep(tc.tile_pool(name="wpp", bufs=1))
        xpool = ep(tc.tile_pool(name="xpool", bufs=10))
        ppool = ep(tc.tile_pool(name="ppool", bufs=15))
        dramp = ep(tc.tile_pool(name="dramp", bufs=1, space="DRAM"))
        mmps = ep(tc.tile_pool(name="mmps", bufs=2, space="PSUM"))

        bias_sb = const.tile([128, 8], f32, name="bias_sb")
        nc.sync.dma_start(out=bias_sb, in_=bpr.rearrange("(j p) -> p j", p=128))

        wcv_sb = [wcvp.tile([128, C], f32r, tag=f"wcv{g}", name=f"wcv{g}")
                  for g in range(8)]
        wp_sb = [wpp.tile([128, C], bf16, tag=f"wp{g}", name=f"wp{g}")
                 for g in range(8)]

        wck_dram = dramp.tile([C, C], f32r, name="wck_dram")

        # ---- merge phase (scoped pools; addresses reused by later pools) ----
        # r-tiles are processed in passes of <=3 so SBUF holds at most 3 a/b
        # pairs per tensor (task can be up to 9 -> rt up to 640)
        with tc.tile_pool(name="mab", bufs=1) as mab, \
                tc.tile_pool(name="mw", bufs=4) as mw:
            passes = [r_tiles[i:i + 3] for i in range(0, len(r_tiles), 3)]
            for is_k, (acat, bcatT, wsrcT) in [
                    (False, (a_v, bvT, wvT)), (True, (a_k, bkT, wkT))]:
                kv = 0 if is_k else 1
                for pi, pr in enumerate(passes):
                    a_sb = []
                    b_sb = []
                    for ri, (r0_, rsz) in enumerate(pr):
                        at = mab.tile([128, C], f32r, tag=f"a{kv}_{ri}",
                                      name=f"a{kv}_{ri}")
                        bt = mab.tile([128, C], f32r, tag=f"b{kv}_{ri}",
                                      name=f"b{kv}_{ri}")
                        nc.sync.dma_start(out=at[0:rsz, :],
                                          in_=acat[r0_:r0_ + rsz, :])
                        nc.sync.dma_start(out=bt[0:rsz, :],
                                          in_=bcatT[r0_:r0_ + rsz, :])
                        a_sb.append(at)
                        b_sb.append(bt)
                    for it in range(8):
                        for cc in range(2):
                            ps = mmps.tile([128, 512], f32, name="ps", tag="ps")
                            for ri, (r0_, rsz) in enumerate(pr):
                                nc.tensor.matmul(
                                    ps[:, :],
                                    lhsT=a_sb[ri][0:rsz, it * 128:(it + 1) * 128],
                                    rhs=b_sb[ri][0:rsz, cc * 512:(cc + 1) * 512],
                                    start=(ri == 0),
                                    stop=(ri == len(pr) - 1),
                                )
                            wt = mw.tile([128, 512], f32, tag="mwsrc", name="mwsrc")
                            if pi == 0:
                                nc.sync.dma_start(
                                    out=wt,
                                    in_=wsrcT[it * 128:(it + 1) * 128,
                                              cc * 512:(cc + 1) * 512])
                            elif kv == 0:
                                nc.sync.dma_start(
                                    out=wt,
                                    in_=wck_dram[it * 128:(it + 1) * 128,
                                                 cc * 512:(cc + 1) * 512]
                                    .bitcast(f32))
                            if kv == 0:
                                mo = mw.tile([128, 512], f32r, tag="mout",
                                             name="mout")
                                nc.vector.tensor_add(mo, ps, wt)
                                nc.sync.dma_start(
                                    out=wck_dram[it * 128:(it + 1) * 128,
                                                 cc * 512:(cc + 1) * 512],
                                    in_=mo)
                            elif pi == 0:
                                nc.vector.tensor_add(
                                    wcv_sb[it][:, cc * 512:(cc + 1) * 512], ps, wt)
                            else:
                                nc.vector.tensor_add(
                                    wcv_sb[it][:, cc * 512:(cc + 1) * 512], ps,
                                    wcv_sb[it][:, cc * 512:(cc + 1) * 512]
                                    .bitcast(f32))

        # ---- remaining pools (reuse merge addresses) ----
        qkpool = ep(tc.tile_pool(name="qkpool", bufs=4))
        vpool = ep(tc.tile_pool(name="vpool", bufs=8))
        wstream = ep(tc.tile_pool(name="wstream", bufs=3))
        otpool = ep(tc.tile_pool(name="otpool", bufs=1))
        avsb = ep(tc.tile_pool(name="avsb", bufs=6))
        bcpool = ep(tc.tile_pool(name="bcpool", bufs=2))
        tmpool = ep(tc.tile_pool(name="tmpool", bufs=2))
        ypool = ep(tc.tile_pool(name="ypool", bufs=2))
        sdpool = ep(tc.tile_pool(name="sdpool", bufs=4, space="DRAM"))
        spsum = ep(tc.tile_pool(name="spsum", bufs=2, space="PSUM"))
        avps = ep(tc.tile_pool(name="avps", bufs=2, space="PSUM"))

        def load_x(b):
            tiles = []
            for g in range(8):
                xt = xpool.tile([128, N], f32r, tag="x", name="x")
                nc.sync.dma_start(
                    out=xt, in_=xT[g * 128:(g + 1) * 128, b * N:(b + 1) * N])
                tiles.append(xt)
            return tiles

        def emit_proj(pb, pot):
            for oc in range(8):
                for nt in range(NT):
                    ps = mmps.tile([128, 512], f32, name="ps", tag="ps")
                    for ct in range(8):
                        nc.tensor.matmul(
                            ps[:, :],
                            lhsT=wp_sb[ct][:, oc * 128:(oc + 1) * 128],
                            rhs=pot[ct][:, nt * 512:(nt + 1) * 512],
                            start=(ct == 0),
                            stop=(ct == 7),
                        )
                    ys = ypool.tile([128, 512], f32, tag="y", name="y")
                    nc.vector.tensor_scalar_add(ys, ps, bias_sb[:, oc:oc + 1])
                    nc.sync.dma_start(
                        out=yT[oc * 128:(oc + 1) * 128,
                               pb * N + nt * 512:pb * N + (nt + 1) * 512],
                        in_=ys)

        prev_proj = None
        xg_next = load_x(0)
        for b in range(BL):
            xg = xg_next

            # --- v projection (token-major, merged weights resident) ---
            vg = [vpool.tile([128, H * 65], bf16, tag="v", name="v")
                  for _ in range(8)]
            for jc in range(2):
                for mt in range(8):
                    ps = mmps.tile([128, 512], f32, name="ps", tag="ps")
                    for ct in range(8):
                        nc.tensor.matmul(
                            ps[:, :],
                            lhsT=xg[ct][:, mt * 128:(mt + 1) * 128],
                            rhs=wcv_sb[ct][:, jc * 512:(jc + 1) * 512],
                            start=(ct == 0),
                            stop=(ct == 7),
                        )
                    vv = vg[mt].rearrange("p (h u) -> p h u", u=65)
                    nc.vector.tensor_copy(
                        out=vv[:, jc * 8:(jc + 1) * 8, 0:64],
                        in_=ps.rearrange("p (h d) -> p h d", d=64))
                    if jc == 0:
                        nc.gpsimd.memset(vv[:, :, 64:65], 1.0)

            if prev_proj is not None:
                emit_proj(*prev_proj)

            # --- q/k projections (feature-major, weights stationary) ---
            # one gathered DMA per (s, g): [p, ct, j] <- W.T[ct*128+p, g*128+j]
            qk_t = {}
            for g in range(8):
                for s in range(2):  # 0 = q, 1 = k(merged)
                    src = wqT if s == 0 else wck_dram
                    wc = wstream.tile([128, N], f32r, tag="wch", name="wch")
                    nc.sync.dma_start(
                        out=wc.rearrange("p (ct j) -> p ct j", j=128),
                        in_=src.rearrange("(ct p) j -> p ct j", p=128)
                        [:, :, g * 128:(g + 1) * 128])
                    t = qkpool.tile([128, N], f32r, tag="qk", name="qk")
                    for nt in range(NT):
                        ps = mmps.tile([128, 512], f32, name="ps", tag="ps")
                        for ct in range(8):
                            nc.tensor.matmul(
                                ps[:, :],
                                lhsT=wc[:, ct * 128:(ct + 1) * 128],
                                rhs=xg[ct][:, nt * 512:(nt + 1) * 512],
                                start=(ct == 0),
                                stop=(ct == 7),
                            )
                        nc.vector.tensor_copy(out=t[:, nt * 512:(nt + 1) * 512], in_=ps)
                    qk_t[(s, g)] = t

            if b == 0:
                for ct in range(8):
                    nc.sync.dma_start(out=wp_sb[ct],
                                      in_=wpT[ct * 128:(ct + 1) * 128, :])
            if b + 1 < BL:
                xg_next = load_x(b + 1)

            # --- attention (head pairs share psum tiles; the two heads of a
            # pair run in different PE row-groups -> concurrent matmuls) ---
            ot = [otpool.tile([128, N], bf16, tag=f"ot{g}", name=f"ot{g}")
                  for g in range(8)]
            for g in range(8):  # head pair (2g, 2g+1)
                qk_q, qk_k = qk_t[(0, g)], qk_t[(1, g)]
                # P tiles: per (mt, nt), cols 0:512 = head 2g, 512:1024 = 2g+1
                pts = {}
                # head 2g's AV accumulation is interleaved with scores/exp so
                # the PE has ready work whenever ACT (exp) is the rate limiter
                chains0 = [avps.tile([128, 512], f32, name="aps", tag="aps")
                           for _ in range(NT)]
                h0 = 2 * g
                for mt in range(8):
                    for nt in range(NT):
                        sps = spsum.tile([128, N], f32, name="sps", tag="sps")
                        for hh in range(2):
                            nc.tensor.matmul(
                                sps[:, hh * 512:(hh + 1) * 512],
                                lhsT=qk_k[hh * 64:(hh + 1) * 64,
                                          mt * 128:(mt + 1) * 128],
                                rhs=qk_q[hh * 64:(hh + 1) * 64,
                                         nt * 512:(nt + 1) * 512],
                            )
                        pt = ppool.tile([128, N], bf16, tag="P", name="P")
                        nc.scalar.activation(out=pt, in_=sps, func=Exp, scale=0.125)
                        pts[(mt, nt)] = pt
                    for nt in range(NT):
                        nc.tensor.matmul(
                            chains0[nt][0:65, :],
                            lhsT=vg[mt][:, h0 * 65:h0 * 65 + 65],
                            rhs=pts[(mt, nt)][:, 0:512],
                            start=(mt == 0),
                            stop=(mt == 7),
                        )
                for hh in range(2):
                    h = 2 * g + hh
                    if hh == 0:
                        chains = chains0
                    else:
                        chains = [avps.tile([128, 512], f32, name="aps",
                                            tag="aps") for _ in range(NT)]
                        for mt in range(8):
                            for nt in range(NT):
                                nc.tensor.matmul(
                                    chains[nt][0:65, :],
                                    lhsT=vg[mt][:, h * 65:h * 65 + 65],
                                    rhs=pts[(mt, nt)][:, 512:1024],
                                    start=(mt == 0),
                                    stop=(mt == 7),
                                )
                    for nt in range(NT):
                        aps = chains[nt]
                        # release PSUM fast: copy [data|sums] to SBUF, then
                        # normalize via reciprocal of a DRAM-bounced partition
                        # broadcast of the sums row.
                        av = avsb.tile([128, 512], f32, tag="av", name="av")
                        nc.vector.tensor_copy(av[0:65, :], aps[0:65, :])
                        sd = sdpool.tile([1, 512], f32, tag="sd", name="sd")
                        nc.sync.dma_start(out=sd, in_=av[64:65, :])
                        bc = bcpool.tile([64, 512], f32, tag="bc", name="bc")
                        nc.sync.dma_start(
                            out=bc, in_=sd[0:1, :].to_broadcast((64, 512)))
                        nc.vector.reciprocal(bc, bc)
                        if hh == 0:
                            nc.vector.tensor_mul(
                                ot[g][0:64, nt * 512:(nt + 1) * 512],
                                av[0:64, :], bc)
                        else:
                            tm = tmpool.tile([64, 512], bf16, tag="tm", name="tm")
                            nc.vector.tensor_mul(tm, av[0:64, :], bc)
                            nc.sync.dma_start(
                                out=ot[g][64:128, nt * 512:(nt + 1) * 512],
                                in_=tm)

            prev_proj = (b, ot)
        emit_proj(*prev_proj)

    nc.finalize()
    return nc


def _prep(x, W_qkv, W_proj, b_proj, A_k, B_k, A_v, B_v, task):
    import ml_dtypes

    tt = int(task)
    rt = HD * (tt + 1)
    f32 = np.float32

    wT = np.ascontiguousarray(W_qkv.astype(f32).T)  # [C, 3C]
    shared = {
        "wqT": np.ascontiguousarray(wT[:, 0:C]),
        "wkT": np.ascontiguousarray(wT[:, C:2 * C]),
        "wvT": np.ascontiguousarray(wT[:, 2 * C:3 * C]),
        "a_k": np.ascontiguousarray(A_k[:tt + 1].astype(f32).reshape(rt, C)),
        "bkT": np.ascontiguousarray(
            B_k[:tt + 1].astype(f32).transpose(0, 2, 1).reshape(rt, C)),
        "a_v": np.ascontiguousarray(A_v[:tt + 1].astype(f32).reshape(rt, C)),
        "bvT": np.ascontiguousarray(
            B_v[:tt + 1].astype(f32).transpose(0, 2, 1).reshape(rt, C)),
        "wpT": np.ascontiguousarray(W_proj.astype(f32).T).astype(ml_dtypes.bfloat16),
        "bpr": np.ascontiguousarray(b_proj.astype(f32)),
    }
    in_maps = []
    for i in range(NCORES):
        xs = x[BL * i:BL * (i + 1)].astype(f32)  # [BL, N, C]
        xt = np.ascontiguousarray(xs.transpose(2, 0, 1).reshape(C, NL))
        in_maps.append({"xT": xt, **shared})
    return rt, in_maps


def kernel(x, W_qkv, W_proj, b_proj, A_k, B_k, A_v, B_v, task):
    from concourse.bass_utils import run_bass_kernel_spmd

    rt, in_maps = _prep(x, W_qkv, W_proj, b_proj, A_k, B_k, A_v, B_v, task)
    if rt not in _BUILT:
        _BUILT[rt] = _build(rt)
    nc = _BUILT[rt]
    res = run_bass_kernel_spmd(nc, in_maps, list(range(NCORES)))
    outs = []
    for i in range(NCORES):
        yt = res.results[i]["yT"]  # [C, NL]
        outs.append(yt.reshape(C, BL, N).transpose(1, 2, 0))
    return np.ascontiguousarray(np.concatenate(outs, axis=0), dtype=np.float32)


# revision 30
# speedup vs baseline: 1.2734x; 1.2734x over previous
"""Trainium2 Bass kernel for InfLoRA attention (self-contained).

Data-parallel over batch across 8 NeuronCores (2 batch elements per core).
Per-core program (all FLOPs on device):
  1. Merge LoRA weights into k/v projection weights:
     w_k_eff.T = W_qkv_k.T + Acat_k.T-contracted product (single matmul chain),
     k result goes to DRAM scratch, v result stays resident in SBUF.
  2. qkv projection in feature-major layout (weights stationary for q/k ->
     feature-major q/k; x stationary for v -> token-major v).
  3. Attention per head pair: scores^T = k^T . q (K=64) with the two heads
     packed into different PE row groups (base partitions 0/64 -> concurrent
     matmuls) sharing one 2-bank PSUM tile; exp on ACT with the 1/sqrt(hd)
     scale folded into the activation's free affine (no max-subtraction:
     |logits| < 4.2); AV matmul in bf16 with a ones-column on v giving the
     softmax denominators for free; normalization via reciprocal + DRAM-bounce
     partition broadcast. Head 0's AV accumulation is interleaved with the
     scores/exp stream so the PE has ready work while ACT is the rate limiter.
  4. Output projection (bf16) + bias, written feature-major and re-transposed
     on the host. Batches are software-pipelined: proj(b-1) is emitted after
     v(b) so batch-b PE work fills batch-(b-1)'s softmax-normalize tail.
"""

import numpy as np

B, N, C = 16, 1024, 1024
H = 16
HD = C // H  # 64
NCORES = 8
BL = B // NCORES  # 2 batch elements per core
NL = BL * N  # 2048 tokens per core
NT = N // 512  # n-tiles per batch

_BUILT = {}


def _build(rt):
    """Build + finalize the per-core Bacc program for RT = 64*(task+1)."""
    from contextlib import ExitStack

    import concourse.tile as tile
    from concourse import bacc, mybir

    f32 = mybir.dt.float32
    bf16 = mybir.dt.bfloat16
    f32r = mybir.dt.float32r
    Exp = mybir.ActivationFunctionType.Exp

    r_tiles = []
    r0 = 0
    while r0 < rt:
        r_tiles.append((r0, min(128, rt - r0)))
        r0 += 128

    nc = bacc.Bacc()

    xT = nc.declare_dram_parameter("xT", [C, NL], f32r, isOutput=False)
    wqT = nc.declare_dram_parameter("wqT", [C, C], f32r, isOutput=False)
    wkT = nc.declare_dram_parameter("wkT", [C, C], f32, isOutput=False)
    wvT = nc.declare_dram_parameter("wvT", [C, C], f32, isOutput=False)
    a_k = nc.declare_dram_parameter("a_k", [rt, C], f32r, isOutput=False)
    bkT = nc.declare_dram_parameter("bkT", [rt, C], f32r, isOutput=False)
    a_v = nc.declare_dram_parameter("a_v", [rt, C], f32r, isOutput=False)
    bvT = nc.declare_dram_parameter("bvT", [rt, C], f32r, isOutput=False)
    wpT = nc.declare_dram_parameter("wpT", [C, C], bf16, isOutput=False)
    bpr = nc.declare_dram_parameter("bpr", [C], f32, isOutput=False)
    yT = nc.declare_dram_parameter("yT", [C, NL], f32, isOutput=True)

    with tile.TileContext(nc) as tc, ExitStack() as ctx:
        ep = ctx.enter_context

        # ---- long-lived pools (created first: low addresses) ----
        const = ep(tc.tile_pool(name="const", bufs=1))
        wcvp = ep(tc.tile_pool(name="wcvp", bufs=1))
        wpp = ep(tc.tile_pool(name="wpp", bufs=1))
        xpool = ep(tc.tile_pool(name="xpool", bufs=10))
        ppool = ep(tc.tile_pool(name="ppool", bufs=15))
        dramp = ep(tc.tile_pool(name="dramp", bufs=1, space="DRAM"))
        mmps = ep(tc.tile_pool(name="mmps", bufs=2, space="PSUM"))

        bias_sb = const.tile([128, 8], f32, name="bias_sb")
        nc.sync.dma_start(out=bias_sb, in_=bpr.rearrange("(j p) -> p j", p=128))

        wcv_sb = [wcvp.tile([128, C], f32r, tag=f"wcv{g}", name=f"wcv{g}")
                  for g in range(8)]
        wp_sb = [wpp.tile([128, C], bf16, tag=f"wp{g}", name=f"wp{g}")
                 for g in range(8)]

        wck_dram = dramp.tile([C, C], f32r, name="wck_dram")

        # ---- merge phase (scoped pools; addresses reused by later pools) ----
        # r-tiles are processed in passes of <=3 so SBUF holds at most 3 a/b
        # pairs per tensor (task can be up to 9 -> rt up to 640)
        with tc.tile_pool(name="mab", bufs=1) as mab, \
                tc.tile_pool(name="mw", bufs=4) as mw:
            passes = [r_tiles[i:i + 3] for i in range(0, len(r_tiles), 3)]
            for is_k, (acat, bcatT, wsrcT) in [
                    (False, (a_v, bvT, wvT)), (True, (a_k, bkT, wkT))]:
                kv = 0 if is_k else 1
                for pi, pr in enumerate(passes):
                    a_sb = []
                    b_sb = []
                    for ri, (r0_, rsz) in enumerate(pr):
                        at = mab.tile([128, C], f32r, tag=f"a{kv}_{ri}",
                                      name=f"a{kv}_{ri}")
                        bt = mab.tile([128, C], f32r, tag=f"b{kv}_{ri}",
                                      name=f"b{kv}_{ri}")
                        nc.sync.dma_start(out=at[0:rsz, :],
                                          in_=acat[r0_:r0_ + rsz, :])
                        nc.sync.dma_start(out=bt[0:rsz, :],
                                          in_=bcatT[r0_:r0_ + rsz, :])
                        a_sb.append(at)
                        b_sb.append(bt)
                    for it in range(8):
                        for cc in range(2):
                            ps = mmps.tile([128, 512], f32, name="ps", tag="ps")
                            for ri, (r0_, rsz) in enumerate(pr):
                                nc.tensor.matmul(
                                    ps[:, :],
                                    lhsT=a_sb[ri][0:rsz, it * 128:(it + 1) * 128],
                                    rhs=b_sb[ri][0:rsz, cc * 512:(cc + 1) * 512],
                                    start=(ri == 0),
                                    stop=(ri == len(pr) - 1),
                                )
                            wt = mw.tile([128, 512], f32, tag="mwsrc", name="mwsrc")
                            if pi == 0:
                                nc.sync.dma_start(
                                    out=wt,
                                    in_=wsrcT[it * 128:(it + 1) * 128,
                                              cc * 512:(cc + 1) * 512])
                            elif kv == 0:
                                nc.sync.dma_start(
                                    out=wt,
                                    in_=wck_dram[it * 128:(it + 1) * 128,
                                                 cc * 512:(cc + 1) * 512]
                                    .bitcast(f32))
                            if kv == 0:
                                mo = mw.tile([128, 512], f32r, tag="mout",
                                             name="mout")
                                nc.vector.tensor_add(mo, ps, wt)
                                nc.sync.dma_start(
                                    out=wck_dram[it * 128:(it + 1) * 128,
                                                 cc * 512:(cc + 1) * 512],
                                    in_=mo)
                            elif pi == 0:
                                nc.vector.tensor_add(
                                    wcv_sb[it][:, cc * 512:(cc + 1) * 512], ps, wt)
                            else:
                                nc.vector.tensor_add(
                                    wcv_sb[it][:, cc * 512:(cc + 1) * 512], ps,
                                    wcv_sb[it][:, cc * 512:(cc + 1) * 512]
                                    .bitcast(f32))

        # ---- remaining pools (reuse merge addresses) ----
        qkpool = ep(tc.tile_pool(name="qkpool", bufs=4))
        vpool = ep(tc.tile_pool(name="vpool", bufs=8))
        wstream = ep(tc.tile_pool(name="wstream", bufs=3))
        otpool = ep(tc.tile_pool(name="otpool", bufs=1))
        avsb = ep(tc.tile_pool(name="avsb", bufs=6))
        bcpool = ep(tc.tile_pool(name="bcpool", bufs=2))
        tmpool = ep(tc.tile_pool(name="tmpool", bufs=2))
        ypool = ep(tc.tile_pool(name="ypool", bufs=2))
        sdpool = ep(tc.tile_pool(name="sdpool", bufs=4, space="DRAM"))
        spsum = ep(tc.tile_pool(name="spsum", bufs=2, space="PSUM"))
        avps = ep(tc.tile_pool(name="avps", bufs=2, space="PSUM"))

        def load_x(b):
            tiles = []
            for g in range(8):
                xt = xpool.tile([128, N], f32r, tag="x", name="x")
                nc.sync.dma_start(
                    out=xt, in_=xT[g * 128:(g + 1) * 128, b * N:(b + 1) * N])
                tiles.append(xt)
            return tiles

        def emit_proj(pb, pot):
            for oc in range(8):
                for nt in range(NT):
                    ps = mmps.tile([128, 512], f32, name="ps", tag="ps")
                    for ct in range(8):
                        nc.tensor.matmul(
                            ps[:, :],
                            lhsT=wp_sb[ct][:, oc * 128:(oc + 1) * 128],
                            rhs=pot[ct][:, nt * 512:(nt + 1) * 512],
                            start=(ct == 0),
                            stop=(ct == 7),
                        )
                    ys = ypool.tile([128, 512], f32, tag="y", name="y")
                    nc.vector.tensor_scalar_add(ys, ps, bias_sb[:, oc:oc + 1])
                    nc.sync.dma_start(
                        out=yT[oc * 128:(oc + 1) * 128,
                               pb * N + nt * 512:pb * N + (nt + 1) * 512],
                        in_=ys)

        prev_proj = None
        xg_next = load_x(0)
        for b in range(BL):
            xg = xg_next

            # --- v projection (token-major, merged weights resident) ---
            vg = [vpool.tile([128, H * 65], bf16, tag="v", name="v")
                  for _ in range(8)]
            for jc in range(2):
                for mt in range(8):
                    ps = mmps.tile([128, 512], f32, name="ps", tag="ps")
                    for ct in range(8):
                        nc.tensor.matmul(
                            ps[:, :],
                            lhsT=xg[ct][:, mt * 128:(mt + 1) * 128],
                            rhs=wcv_sb[ct][:, jc * 512:(jc + 1) * 512],
                            start=(ct == 0),
                            stop=(ct == 7),
                        )
                    vv = vg[mt].rearrange("p (h u) -> p h u", u=65)
                    nc.vector.tensor_copy(
                        out=vv[:, jc * 8:(jc + 1) * 8, 0:64],
                        in_=ps.rearrange("p (h d) -> p h d", d=64))
                    if jc == 0:
                        nc.gpsimd.memset(vv[:, :, 64:65], 1.0)

            if prev_proj is not None:
                emit_proj(*prev_proj)

            # --- q/k projections (feature-major, weights stationary) ---
            # one gathered DMA per (s, g): [p, ct, j] <- W.T[ct*128+p, g*128+j]
            qk_t = {}
            for g in range(8):
                for s in range(2):  # 0 = q, 1 = k(merged)
                    src = wqT if s == 0 else wck_dram
                    wc = wstream.tile([128, N], f32r, tag="wch", name="wch")
                    nc.sync.dma_start(
                        out=wc.rearrange("p (ct j) -> p ct j", j=128),
                        in_=src.rearrange("(ct p) j -> p ct j", p=128)
                        [:, :, g * 128:(g + 1) * 128])
                    t = qkpool.tile([128, N], f32r, tag="qk", name="qk")
                    for nt in range(NT):
                        ps = mmps.tile([128, 512], f32, name="ps", tag="ps")
                        for ct in range(8):
                            nc.tensor.matmul(
                                ps[:, :],
                                lhsT=wc[:, ct * 128:(ct + 1) * 128],
                                rhs=xg[ct][:, nt * 512:(nt + 1) * 512],
                                start=(ct == 0),
                                stop=(ct == 7),
                            )
                        nc.vector.tensor_copy(out=t[:, nt * 512:(nt + 1) * 512], in_=ps)
                    qk_t[(s, g)] = t

            if b == 0:
                for ct in range(8):
                    nc.sync.dma_start(out=wp_sb[ct],
                                      in_=wpT[ct * 128:(ct + 1) * 128, :])
            if b + 1 < BL:
                xg_next = load_x(b + 1)

            # --- attention (head pairs share psum tiles; the two heads of a
            # pair run in different PE row-groups -> concurrent matmuls) ---
            ot = [otpool.tile([128, N], bf16, tag=f"ot{g}", name=f"ot{g}")
                  for g in range(8)]
            for g in range(8):  # head pair (2g, 2g+1)
                qk_q, qk_k = qk_t[(0, g)], qk_t[(1, g)]
                # P tiles: per (mt, nt), cols 0:512 = head 2g, 512:1024 = 2g+1
                pts = {}
                # head 2g's AV accumulation is interleaved with scores/exp so
                # the PE has ready work whenever ACT (exp) is the rate limiter
                chains0 = [avps.tile([128, 512], f32, name="aps", tag="aps")
                           for _ in range(NT)]
                h0 = 2 * g
                for mt in range(8):
                    for nt in range(NT):
                        sps = spsum.tile([128, N], f32, name="sps", tag="sps")
                        for hh in range(2):
                            nc.tensor.matmul(
                                sps[:, hh * 512:(hh + 1) * 512],
                                lhsT=qk_k[hh * 64:(hh + 1) * 64,
                                          mt * 128:(mt + 1) * 128],
                                rhs=qk_q[hh * 64:(hh + 1) * 64,
                                         nt * 512:(nt + 1) * 512],
                            )
                        pt = ppool.tile([128, N], bf16, tag="P", name="P")
                        nc.scalar.activation(out=pt, in_=sps, func=Exp, scale=0.125)
                        pts[(mt, nt)] = pt
                    for nt in range(NT):
                        nc.tensor.matmul(
                            chains0[nt][0:65, :],
                            lhsT=vg[mt][:, h0 * 65:h0 * 65 + 65],
                            rhs=pts[(mt, nt)][:, 0:512],
                            start=(mt == 0),
                            stop=(mt == 7),
                        )
                for hh in range(2):
                    h = 2 * g + hh
                    if hh == 0:
                        chains = chains0
                    else:
                        chains = [avps.tile([128, 512], f32, name="aps",
                                            tag="aps") for _ in range(NT)]
                        for mt in range(8):
                            for nt in range(NT):
                                nc.tensor.matmul(
                                    chains[nt][0:65, :],
                                    lhsT=vg[mt][:, h * 65:h * 65 + 65],
                                    rhs=pts[(mt, nt)][:, 512:1024],
                                    start=(mt == 0),
                                    stop=(mt == 7),
                                )
                    for nt in range(NT):
                        aps = chains[nt]
                        # release PSUM fast: copy [data|sums] to SBUF, then
                        # normalize via reciprocal of a DRAM-bounced partition
                        # broadcast of the sums row.
                        av = avsb.tile([128, 512], f32, tag="av", name="av")
                        nc.vector.tensor_copy(av[0:65, :], aps[0:65, :])
                        sd = sdpool.tile([1, 512], f32, tag="sd", name="sd")
                        nc.sync.dma_start(out=sd, in_=av[64:65, :])
                        bc = bcpool.tile([64, 512], f32, tag="bc", name="bc")
                        nc.sync.dma_start(
                            out=bc, in_=sd[0:1, :].to_broadcast((64, 512)))
                        nc.vector.reciprocal(bc, bc)
                        if hh == 0:
                            nc.vector.tensor_mul(
                                ot[g][0:64, nt * 512:(nt + 1) * 512],
                                av[0:64, :], bc)
                        else:
                            tm = tmpool.tile([64, 512], bf16, tag="tm", name="tm")
                            nc.vector.tensor_mul(tm, av[0:64, :], bc)
                            nc.sync.dma_start(
                                out=ot[g][64:128, nt * 512:(nt + 1) * 512],
                                in_=tm)

            prev_proj = (b, ot)
        emit_proj(*prev_proj)

    nc.finalize()
    return nc


def _prep(x, W_qkv, W_proj, b_proj, A_k, B_k, A_v, B_v, task):
    import ml_dtypes

    tt = int(task)
    rt = HD * (tt + 1)
    f32 = np.float32

    wT = np.ascontiguousarray(W_qkv.astype(f32).T)  # [C, 3C]
    shared = {
        "wqT": np.ascontiguousarray(wT[:, 0:C]),
        "wkT": np.ascontiguousarray(wT[:, C:2 * C]),
        "wvT": np.ascontiguousarray(wT[:, 2 * C:3 * C]),
        "a_k": np.ascontiguousarray(A_k[:tt + 1].astype(f32).reshape(rt, C)),
        "bkT": np.ascontiguousarray(
            B_k[:tt + 1].astype(f32).transpose(0, 2, 1).reshape(rt, C)),
        "a_v": np.ascontiguousarray(A_v[:tt + 1].astype(f32).reshape(rt, C)),
        "bvT": np.ascontiguousarray(
            B_v[:tt + 1].astype(f32).transpose(0, 2, 1).reshape(rt, C)),
        "wpT": np.ascontiguousarray(W_proj.astype(f32).T).astype(ml_dtypes.bfloat16),
        "bpr": np.ascontiguousarray(b_proj.astype(f32)),
    }
    in_maps = []
    for i in range(NCORES):
        xs = x[BL * i:BL * (i + 1)].astype(f32)  # [BL, N, C]
        xt = np.ascontiguousarray(xs.transpose(2, 0, 1).reshape(C, NL))
        in_maps.append({"xT": xt, **shared})
    return rt, in_maps


def kernel(x, W_qkv, W_proj, b_proj, A_k, B_k, A_v, B_v, task):
    from concourse.bass_utils import run_bass_kernel_spmd

    rt, in_maps = _prep(x, W_qkv, W_proj, b_proj, A_k, B_k, A_v, B_v, task)
    if rt not in _BUILT:
        _BUILT[rt] = _build(rt)
    nc = _BUILT[rt]
    res = run_bass_kernel_spmd(nc, in_maps, list(range(NCORES)))
    outs = []
    for i in range(NCORES):
        yt = res.results[i]["yT"]  # [C, NL]
        outs.append(yt.reshape(C, BL, N).transpose(1, 2, 0))
    return np.ascontiguousarray(np.concatenate(outs, axis=0), dtype=np.float32)


# revision 34
# speedup vs baseline: 4.0052x; 3.1453x over previous
"""Trainium2 Bass kernel for InfLoRA attention (self-contained).

Data-parallel over batch across 8 NeuronCores (2 batch elements per core).
Per-core program (all FLOPs on device):
  1. Merge LoRA weights into k/v projection weights:
     w_k_eff.T = W_qkv_k.T + Acat_k.T-contracted product (single matmul chain),
     k result goes to DRAM scratch, v result stays resident in SBUF.
  2. qkv projection in feature-major layout (weights stationary for q/k ->
     feature-major q/k; x stationary for v -> token-major v).
  3. Attention per head pair: scores^T = k^T . q (K=64) with the two heads
     packed into different PE row groups (base partitions 0/64 -> concurrent
     matmuls) sharing one 2-bank PSUM tile; exp on ACT with the 1/sqrt(hd)
     scale folded into the activation's free affine (no max-subtraction:
     |logits| < 4.2); AV matmul in bf16 with a ones-column on v giving the
     softmax denominators for free; normalization via reciprocal + DRAM-bounce
     partition broadcast. Head 0's AV accumulation is interleaved with the
     scores/exp stream so the PE has ready work while ACT is the rate limiter.
  4. Output projection (bf16) + bias, written feature-major and re-transposed
     on the host. Batches are software-pipelined: proj(b-1) is emitted after
     v(b) so batch-b PE work fills batch-(b-1)'s softmax-normalize tail.
"""

import numpy as np

B, N, C = 16, 1024, 1024
H = 16
HD = C // H  # 64
NCORES = 8
BL = B // NCORES  # 2 batch elements per core
NL = BL * N  # 2048 tokens per core
NT = N // 512  # n-tiles per batch

_BUILT = {}


def _build(rt):
    """Build + finalize the per-core Bacc program for RT = 64*(task+1)."""
    from contextlib import ExitStack

    import concourse.tile as tile
    from concourse import bacc, mybir

    f32 = mybir.dt.float32
    bf16 = mybir.dt.bfloat16
    f32r = mybir.dt.float32r
    Exp = mybir.ActivationFunctionType.Exp

    r_tiles = []
    r0 = 0
    while r0 < rt:
        r_tiles.append((r0, min(128, rt - r0)))
        r0 += 128

    nc = bacc.Bacc()

    xT = nc.declare_dram_parameter("xT", [C, NL], f32r, isOutput=False)
    wqT = nc.declare_dram_parameter("wqT", [C, C], f32r, isOutput=False)
    wkT = nc.declare_dram_parameter("wkT", [C, C], f32, isOutput=False)
    wvT = nc.declare_dram_parameter("wvT", [C, C], f32, isOutput=False)
    a_k = nc.declare_dram_parameter("a_k", [rt, C], f32r, isOutput=False)
    bkT = nc.declare_dram_parameter("bkT", [rt, C], f32r, isOutput=False)
    a_v = nc.declare_dram_parameter("a_v", [rt, C], f32r, isOutput=False)
    bvT = nc.declare_dram_parameter("bvT", [rt, C], f32r, isOutput=False)
    wpT = nc.declare_dram_parameter("wpT", [C, C], bf16, isOutput=False)
    bpr = nc.declare_dram_parameter("bpr", [C], f32, isOutput=False)
    yT = nc.declare_dram_parameter("yT", [C, NL], f32, isOutput=True)

    with tile.TileContext(nc) as tc, ExitStack() as ctx:
        ep = ctx.enter_context

        # ---- long-lived pools (created first: low addresses) ----
        const = ep(tc.tile_pool(name="const", bufs=1))
        wcvp = ep(tc.tile_pool(name="wcvp", bufs=1))
        wpp = ep(tc.tile_pool(name="wpp", bufs=1))
        xpool = ep(tc.tile_pool(name="xpool", bufs=10))
        ppool = ep(tc.tile_pool(name="ppool", bufs=15))
        dramp = ep(tc.tile_pool(name="dramp", bufs=1, space="DRAM"))
        mmps = ep(tc.tile_pool(name="mmps", bufs=2, space="PSUM"))

        bias_sb = const.tile([128, 8], f32, name="bias_sb")
        nc.sync.dma_start(out=bias_sb, in_=bpr.rearrange("(j p) -> p j", p=128))

        wcv_sb = [wcvp.tile([128, C], f32r, tag=f"wcv{g}", name=f"wcv{g}")
                  for g in range(8)]
        wp_sb = [wpp.tile([128, C], bf16, tag=f"wp{g}", name=f"wp{g}")
                 for g in range(8)]

        wck_dram = dramp.tile([C, C], f32r, name="wck_dram")

        # ---- merge phase (scoped pools; addresses reused by later pools) ----
        # r-tiles are processed in passes of <=3 so SBUF holds at most 3 a/b
        # pairs per tensor (task can be up to 9 -> rt up to 640)
        with tc.tile_pool(name="mab", bufs=1) as mab, \
                tc.tile_pool(name="mw", bufs=4) as mw:
            passes = [r_tiles[i:i + 3] for i in range(0, len(r_tiles), 3)]
            for is_k, (acat, bcatT, wsrcT) in [
                    (False, (a_v, bvT, wvT)), (True, (a_k, bkT, wkT))]:
                kv = 0 if is_k else 1
                for pi, pr in enumerate(passes):
                    a_sb = []
                    b_sb = []
                    for ri, (r0_, rsz) in enumerate(pr):
                        at = mab.tile([128, C], f32r, tag=f"a{kv}_{ri}",
                                      name=f"a{kv}_{ri}")
                        bt = mab.tile([128, C], f32r, tag=f"b{kv}_{ri}",
                                      name=f"b{kv}_{ri}")
                        nc.sync.dma_start(out=at[0:rsz, :],
                                          in_=acat[r0_:r0_ + rsz, :])
                        nc.sync.dma_start(out=bt[0:rsz, :],
                                          in_=bcatT[r0_:r0_ + rsz, :])
                        a_sb.append(at)
                        b_sb.append(bt)
                    for it in range(8):
                        for cc in range(2):
                            ps = mmps.tile([128, 512], f32, name="ps", tag="ps")
                            for ri, (r0_, rsz) in enumerate(pr):
                                nc.tensor.matmul(
                                    ps[:, :],
                                    lhsT=a_sb[ri][0:rsz, it * 128:(it + 1) * 128],
                                    rhs=b_sb[ri][0:rsz, cc * 512:(cc + 1) * 512],
                                    start=(ri == 0),
                                    stop=(ri == len(pr) - 1),
                                )
                            wt = mw.tile([128, 512], f32, tag="mwsrc", name="mwsrc")
                            if pi == 0:
                                nc.sync.dma_start(
                                    out=wt,
                                    in_=wsrcT[it * 128:(it + 1) * 128,
                                              cc * 512:(cc + 1) * 512])
                            elif kv == 0:
                                nc.sync.dma_start(
                                    out=wt,
                                    in_=wck_dram[it * 128:(it + 1) * 128,
                                                 cc * 512:(cc + 1) * 512]
                                    .bitcast(f32))
                            if kv == 0:
                                mo = mw.tile([128, 512], f32r, tag="mout",
                                             name="mout")
                                nc.vector.tensor_add(mo, ps, wt)
                                nc.sync.dma_start(
                                    out=wck_dram[it * 128:(it + 1) * 128,
                                                 cc * 512:(cc + 1) * 512],
                                    in_=mo)
                            elif pi == 0:
                                nc.vector.tensor_add(
                                    wcv_sb[it][:, cc * 512:(cc + 1) * 512], ps, wt)
                            else:
                                nc.vector.tensor_add(
                                    wcv_sb[it][:, cc * 512:(cc + 1) * 512], ps,
                                    wcv_sb[it][:, cc * 512:(cc + 1) * 512]
                                    .bitcast(f32))

        # ---- remaining pools (reuse merge addresses) ----
        qkpool = ep(tc.tile_pool(name="qkpool", bufs=4))
        vpool = ep(tc.tile_pool(name="vpool", bufs=8))
        wstream = ep(tc.tile_pool(name="wstream", bufs=3))
        otpool = ep(tc.tile_pool(name="otpool", bufs=1))
        avsb = ep(tc.tile_pool(name="avsb", bufs=6))
        bcpool = ep(tc.tile_pool(name="bcpool", bufs=2))
        tmpool = ep(tc.tile_pool(name="tmpool", bufs=2))
        ypool = ep(tc.tile_pool(name="ypool", bufs=2))
        sdpool = ep(tc.tile_pool(name="sdpool", bufs=4, space="DRAM"))
        spsum = ep(tc.tile_pool(name="spsum", bufs=2, space="PSUM"))
        avps = ep(tc.tile_pool(name="avps", bufs=2, space="PSUM"))

        def load_x(b):
            tiles = []
            for g in range(8):
                xt = xpool.tile([128, N], f32r, tag="x", name="x")
                nc.sync.dma_start(
                    out=xt, in_=xT[g * 128:(g + 1) * 128, b * N:(b + 1) * N])
                tiles.append(xt)
            return tiles

        def emit_proj(pb, pot):
            for oc in range(8):
                for nt in range(NT):
                    ps = mmps.tile([128, 512], f32, name="ps", tag="ps")
                    for ct in range(8):
                        nc.tensor.matmul(
                            ps[:, :],
                            lhsT=wp_sb[ct][:, oc * 128:(oc + 1) * 128],
                            rhs=pot[ct][:, nt * 512:(nt + 1) * 512],
                            start=(ct == 0),
                            stop=(ct == 7),
                        )
                    ys = ypool.tile([128, 512], f32, tag="y", name="y")
                    nc.vector.tensor_scalar_add(ys, ps, bias_sb[:, oc:oc + 1])
                    nc.sync.dma_start(
                        out=yT[oc * 128:(oc + 1) * 128,
                               pb * N + nt * 512:pb * N + (nt + 1) * 512],
                        in_=ys)

        prev_proj = None
        xg_next = load_x(0)
        for b in range(BL):
            xg = xg_next

            # --- v projection (token-major, merged weights resident) ---
            vg = [vpool.tile([128, H * 65], bf16, tag="v", name="v")
                  for _ in range(8)]
            for jc in range(2):
                for mt in range(8):
                    ps = mmps.tile([128, 512], f32, name="ps", tag="ps")
                    for ct in range(8):
                        nc.tensor.matmul(
                            ps[:, :],
                            lhsT=xg[ct][:, mt * 128:(mt + 1) * 128],
                            rhs=wcv_sb[ct][:, jc * 512:(jc + 1) * 512],
                            start=(ct == 0),
                            stop=(ct == 7),
                        )
                    vv = vg[mt].rearrange("p (h u) -> p h u", u=65)
                    nc.vector.tensor_copy(
                        out=vv[:, jc * 8:(jc + 1) * 8, 0:64],
                        in_=ps.rearrange("p (h d) -> p h d", d=64))
                    if jc == 0:
                        nc.gpsimd.memset(vv[:, :, 64:65], 1.0)

            if prev_proj is not None:
                emit_proj(*prev_proj)

            # --- q/k projections (feature-major, weights stationary) ---
            # one gathered DMA per (s, g): [p, ct, j] <- W.T[ct*128+p, g*128+j]
            qk_t = {}
            for g in range(8):
                for s in range(2):  # 0 = q, 1 = k(merged)
                    src = wqT if s == 0 else wck_dram
                    wc = wstream.tile([128, N], f32r, tag="wch", name="wch")
                    nc.sync.dma_start(
                        out=wc.rearrange("p (ct j) -> p ct j", j=128),
                        in_=src.rearrange("(ct p) j -> p ct j", p=128)
                        [:, :, g * 128:(g + 1) * 128])
                    t = qkpool.tile([128, N], f32r, tag="qk", name="qk")
                    for nt in range(NT):
                        ps = mmps.tile([128, 512], f32, name="ps", tag="ps")
                        for ct in range(8):
                            nc.tensor.matmul(
                                ps[:, :],
                                lhsT=wc[:, ct * 128:(ct + 1) * 128],
                                rhs=xg[ct][:, nt * 512:(nt + 1) * 512],
                                start=(ct == 0),
                                stop=(ct == 7),
                            )
                        nc.vector.tensor_copy(out=t[:, nt * 512:(nt + 1) * 512], in_=ps)
                    qk_t[(s, g)] = t

            if b == 0:
                for ct in range(8):
                    nc.sync.dma_start(out=wp_sb[ct],
                                      in_=wpT[ct * 128:(ct + 1) * 128, :])
            if b + 1 < BL:
                xg_next = load_x(b + 1)

            # --- attention (head pairs share psum tiles; the two heads of a
            # pair run in different PE row-groups -> concurrent matmuls) ---
            ot = [otpool.tile([128, N], bf16, tag=f"ot{g}", name=f"ot{g}")
                  for g in range(8)]
            for g in range(8):  # head pair (2g, 2g+1)
                qk_q, qk_k = qk_t[(0, g)], qk_t[(1, g)]
                # P tiles: per (mt, nt), cols 0:512 = head 2g, 512:1024 = 2g+1
                pts = {}
                # head 2g's AV accumulation is interleaved with scores/exp so
                # the PE has ready work whenever ACT (exp) is the rate limiter
                chains0 = [avps.tile([128, 512], f32, name="aps", tag="aps")
                           for _ in range(NT)]
                h0 = 2 * g
                for mt in range(8):
                    for nt in range(NT):
                        sps = spsum.tile([128, N], f32, name="sps", tag="sps")
                        for hh in range(2):
                            nc.tensor.matmul(
                                sps[:, hh * 512:(hh + 1) * 512],
                                lhsT=qk_k[hh * 64:(hh + 1) * 64,
                                          mt * 128:(mt + 1) * 128],
                                rhs=qk_q[hh * 64:(hh + 1) * 64,
                                         nt * 512:(nt + 1) * 512],
                            )
                        pt = ppool.tile([128, N], bf16, tag="P", name="P")
                        nc.scalar.activation(out=pt, in_=sps, func=Exp, scale=0.125)
                        pts[(mt, nt)] = pt
                    for nt in range(NT):
                        nc.tensor.matmul(
                            chains0[nt][0:65, :],
                            lhsT=vg[mt][:, h0 * 65:h0 * 65 + 65],
                            rhs=pts[(mt, nt)][:, 0:512],
                            start=(mt == 0),
                            stop=(mt == 7),
                        )
                for hh in range(2):
                    h = 2 * g + hh
                    if hh == 0:
                        chains = chains0
                    else:
                        chains = [avps.tile([128, 512], f32, name="aps",
                                            tag="aps") for _ in range(NT)]
                        for mt in range(8):
                            for nt in range(NT):
                                nc.tensor.matmul(
                                    chains[nt][0:65, :],
                                    lhsT=vg[mt][:, h * 65:h * 65 + 65],
                                    rhs=pts[(mt, nt)][:, 512:1024],
                                    start=(mt == 0),
                                    stop=(mt == 7),
                                )
                    for nt in range(NT):
                        aps = chains[nt]
                        # release PSUM fast: copy [data|sums] to SBUF, then
                        # normalize via reciprocal of a DRAM-bounced partition
                        # broadcast of the sums row.
                        av = avsb.tile([128, 512], f32, tag="av", name="av")
                        nc.vector.tensor_copy(av[0:65, :], aps[0:65, :])
                        sd = sdpool.tile([1, 512], f32, tag="sd", name="sd")
                        nc.sync.dma_start(out=sd, in_=av[64:65, :])
                        bc = bcpool.tile([64, 512], f32, tag="bc", name="bc")
                        nc.sync.dma_start(
                            out=bc, in_=sd[0:1, :].to_broadcast((64, 512)))
                        nc.vector.reciprocal(bc, bc)
                        if hh == 0:
                            nc.vector.tensor_mul(
                                ot[g][0:64, nt * 512:(nt + 1) * 512],
                                av[0:64, :], bc)
                        else:
                            tm = tmpool.tile([64, 512], bf16, tag="tm", name="tm")
                            nc.vector.tensor_mul(tm, av[0:64, :], bc)
                            nc.sync.dma_start(
                                out=ot[g][64:128, nt * 512:(nt + 1) * 512],
                                in_=tm)

            prev_proj = (b, ot)
        emit_proj(*prev_proj)

    nc.finalize()
    return nc


def _prep(x, W_qkv, W_proj, b_proj, A_k, B_k, A_v, B_v, task):
    import ml_dtypes

    tt = int(task)
    rt = HD * (tt + 1)
    f32 = np.float32

    wT = np.ascontiguousarray(W_qkv.astype(f32).T)  # [C, 3C]
    shared = {
        "wqT": np.ascontiguousarray(wT[:, 0:C]),
        "wkT": np.ascontiguousarray(wT[:, C:2 * C]),
        "wvT": np.ascontiguousarray(wT[:, 2 * C:3 * C]),
        "a_k": np.ascontiguousarray(A_k[:tt + 1].astype(f32).reshape(rt, C)),
        "bkT": np.ascontiguousarray(
            B_k[:tt + 1].astype(f32).transpose(0, 2, 1).reshape(rt, C)),
        "a_v": np.ascontiguousarray(A_v[:tt + 1].astype(f32).reshape(rt, C)),
        "bvT": np.ascontiguousarray(
            B_v[:tt + 1].astype(f32).transpose(0, 2, 1).reshape(rt, C)),
        "wpT": np.ascontiguousarray(W_proj.astype(f32).T).astype(ml_dtypes.bfloat16),
        "bpr": np.ascontiguousarray(b_proj.astype(f32)),
    }
    in_maps = []
    for i in range(NCORES):
        xs = x[BL * i:BL * (i + 1)].astype(f32)  # [BL, N, C]
        xt = np.ascontiguousarray(xs.transpose(2, 0, 1).reshape(C, NL))
        in_maps.append({"xT": xt, **shared})
    return rt, in_maps


def kernel(x, W_qkv, W_proj, b_proj, A_k, B_k, A_v, B_v, task):
    from concourse.bass_utils import run_bass_kernel_spmd

    rt, in_maps = _prep(x, W_qkv, W_proj, b_proj, A_k, B_k, A_v, B_v, task)
    if rt not in _BUILT:
        _BUILT[rt] = _build(rt)
    nc = _BUILT[rt]
    res = run_bass_kernel_spmd(nc, in_maps, list(range(NCORES)))
    outs = []
    for i in range(NCORES):
        yt = res.results[i]["yT"]  # [C, NL]
        outs.append(yt.reshape(C, BL, N).transpose(1, 2, 0))
    return np.ascontiguousarray(np.concatenate(outs, axis=0), dtype=np.float32)
